# revision 17
# baseline (speedup 1.0000x reference)
"""Trainium2 Bass kernel for the UR5e reflected-mass cost function.

Closed-form math (per sample n of 131072 = 2048 b x 64 h):
  The last joint (q6) never affects the output (its Jacobian column is 0),
  and in the q1-rotated "cylindrical" frame every frame origin is
  p_i = (A_i, B_i, C_i) with the z-axes {z0=ez, z1=z2=z3=(0,1,0),
  z4=(s234,0,-c234)}.  All Jacobian columns, the 5x5 mass matrix, and the
  end-effector direction reduce to ~260 scalar ops instead of the naive
  ~670 of the frame-by-frame DH chain.

Implementation: every per-sample scalar is a [128,128] f32 SBUF tile
(16384 samples per core, 8 cores data-parallel over b).  The computation
is a symbolic scalar DAG with CSE + constant folding + STT fusion,
scheduled onto the DVE/ACT/GPSIMD engines with an earliest-finish-time
list scheduler and emitted through the Tile framework.
"""

import math
import numpy as np

# ----------------------------------------------------------------------------
# constants
# ----------------------------------------------------------------------------

PI = math.pi
A2C, A3C = -0.425, -0.3922
D1, D4, D5, D6 = 0.1625, 0.1333, 0.0997, 0.0996
# LINK_MASS[i] sits at frame origin p_{i+1}; link 0 (at p1) never moves.
M1, M2, M3, M4, M5 = 8.058, 2.846, 1.37, 1.3, 0.365
M23 = M2 + M3
M45 = M4 + M5
ROTOR = 0.1
MAGIC = 12582912.0  # 1.5 * 2**23 f32 round-to-int trick

# host channel order handed to the device
# 0:q2 1:q3 2:q4 3:q1 4:q5 5:hx 6:hy 7:hz
SRC_COLS = [7, 8, 9, 6, 10, 19, 20, 21]

# ----------------------------------------------------------------------------
# symbolic scalar DAG
# ----------------------------------------------------------------------------


class Expr:
    __slots__ = ("op", "args", "c", "id", "users", "engine", "fused_into",
                 "slot", "order", "prio", "start", "finish")

    def __init__(self, op, args=(), c=None, i=0):
        self.op = op
        self.args = args
        self.c = c
        self.id = i
        self.users = []
        self.engine = None
        self.fused_into = None
        self.slot = None
        self.order = None
        self.prio = 0.0
        self.start = 0.0
        self.finish = 0.0


class Graph:
    def __init__(self):
        self.nodes = []
        self.cse = {}

    def _mk(self, op, args=(), c=None):
        key = (op, tuple(a.id for a in args), c)
        n = self.cse.get(key)
        if n is None:
            n = Expr(op, args, c, len(self.nodes))
            self.nodes.append(n)
            self.cse[key] = n
        return n

    def C(self, v):
        return self._mk("const", c=float(v))

    def IN(self, ch):
        return self._mk("in", c=ch)

    def add(self, x, y):
        if x.op == "const" and y.op == "const":
            return self.C(x.c + y.c)
        if x.op == "const":
            x, y = y, x
        if y.op == "const":
            if y.c == 0.0:
                return x
            return self._mk("cadd", (x,), y.c)
        a, b = (x, y) if x.id <= y.id else (y, x)
        return self._mk("add", (a, b))

    def sub(self, x, y):
        if x.op == "const" and y.op == "const":
            return self.C(x.c - y.c)
        if y.op == "const":
            if y.c == 0.0:
                return x
            return self._mk("cadd", (x,), -y.c)
        if x.op == "const" and x.c == 0.0:
            return self.cmul(-1.0, y)
        if x is y:
            return self.C(0.0)
        return self._mk("sub", (x, y))

    def cmul(self, c, x):
        c = float(c)
        if x.op == "const":
            return self.C(c * x.c)
        if c == 0.0:
            return self.C(0.0)
        if c == 1.0:
            return x
        if x.op == "cmul":
            return self.cmul(c * x.c, x.args[0])
        return self._mk("cmul", (x,), c)

    def mul(self, x, y):
        if x.op == "const":
            return self.cmul(x.c, y)
        if y.op == "const":
            return self.cmul(y.c, x)
        if x.op == "cmul" and y.op == "cmul":
            return self.cmul(x.c * y.c, self.mul(x.args[0], y.args[0]))
        if x.op == "cmul":
            return self.cmul(x.c, self.mul(x.args[0], y))
        if y.op == "cmul":
            return self.cmul(y.c, self.mul(x, y.args[0]))
        if x is y:
            return self._mk("square", (x,))
        a, b = (x, y) if x.id <= y.id else (y, x)
        return self._mk("mul", (a, b))

    def ts2(self, x, s1, op0, s2, op1):
        return self._mk("ts2", (x,), (float(s1), op0, float(s2), op1))

    def trig(self, q, phase):
        """sin(q + phase), range-reduced so the Sin input is in [-pi, pi]."""
        inv2pi = 1.0 / (2.0 * PI)
        if phase == 0.0:
            t1 = self.ts2(q, inv2pi, "mult", MAGIC, "add")
            k = self._mk("cadd", (t1,), -MAGIC)
        else:
            t0 = self.ts2(q, inv2pi, "mult", phase * inv2pi, "add")
            t1 = self._mk("cadd", (t0,), MAGIC)
            k = self._mk("cadd", (t1,), -MAGIC)
        r0 = self.add(self.cmul(-2.0 * PI, k), q)  # fuses to one STT
        return self._mk("sin", (r0,), (1.0, float(phase)))

    def sqrt_(self, x):
        return self._mk("sqrt", (x,))

    def recip(self, x):
        return self._mk("recip", (x,))

    def sq(self, x):
        return self._mk("square", (x,))


def build_graph():
    """Returns (graph, cost_neg_node). cost_neg = -cost per sample."""
    g = Graph()
    q2, q3, q4, q1, q5 = (g.IN(i) for i in range(5))
    hx, hy, hz = (g.IN(5 + i) for i in range(3))

    q23 = g.add(q2, q3)
    q234 = g.add(q23, q4)
    s1, c1 = g.trig(q1, 0.0), g.trig(q1, PI / 2)
    s2, c2 = g.trig(q2, 0.0), g.trig(q2, PI / 2)
    s23, c23 = g.trig(q23, 0.0), g.trig(q23, PI / 2)
    s234, c234 = g.trig(q234, 0.0), g.trig(q234, PI / 2)
    s5, c5 = g.trig(q5, 0.0), g.trig(q5, PI / 2)

    # cylindrical coordinates (relative: A1 = K1 = 0, K = C - d1)
    A2 = g.cmul(A2C, c2)
    E = g.add(A2, g.cmul(A3C, c23))
    K2 = g.cmul(A2C, s2)
    K3 = g.add(K2, g.cmul(A3C, s23))
    cc = g.mul(c234, s5)
    sc_ = g.mul(s234, s5)
    c45 = g.mul(c234, c5)
    s45 = g.mul(s234, c5)
    A5 = g.add(E, g.cmul(D5, s234))
    A6 = g.sub(A5, g.cmul(D6, cc))
    K5 = g.sub(K3, g.cmul(D5, c234))
    K6 = g.sub(K5, g.cmul(D6, sc_))
    B6 = g.ts2(c5, D6, "mult", D4, "add")   # B6 = d4 + d6*c5

    # squares
    A2s, Es, A5s, A6s = g.sq(A2), g.sq(E), g.sq(A5), g.sq(A6)
    K2s, K3s, K5s, K6s = g.sq(K2), g.sq(K3), g.sq(K5), g.sq(K6)
    B6s = g.sq(B6)

    # weighted square sums (suffix style so S45 comes free)
    SA45 = g.add(g.cmul(M5, A6s), g.cmul(M4, A5s))
    SA = g.add(g.add(SA45, g.cmul(M23, Es)), g.cmul(M1, A2s))
    SK45 = g.add(g.cmul(M5, K6s), g.cmul(M4, K5s))
    SK = g.add(g.add(SK45, g.cmul(M23, K3s)), g.cmul(M1, K2s))
    M11nr = g.add(SA, SK)
    M11 = g.add(M11nr, g.C(ROTOR))
    M00 = g.add(g.add(SA, g.cmul(M5, B6s)), g.C((M3 + M4) * D4 * D4 + ROTOR))
    S45 = g.add(SA45, SK45)

    # weighted linear sums
    WK2 = g.add(g.cmul(M4, K5), g.cmul(M5, K6))
    WK = g.add(g.cmul(M23, K3), WK2)
    WA2 = g.add(g.cmul(M4, A5), g.cmul(M5, A6))
    WA = g.add(g.cmul(M23, E), WA2)

    # M row 0 (joint 1 uses (B, A) plane)
    bk6 = g.mul(B6, K6)
    bk2 = g.mul(B6, K2)
    k63 = g.sub(K6, K3)
    M01 = g.add(g.add(g.cmul(-M3 * D4, K3), g.cmul(-M4 * D4, K5)),
                g.cmul(-M5, bk6))
    M02 = g.add(g.add(M01, g.cmul((M3 + M4) * D4, K2)), g.cmul(M5, bk2))
    M03 = g.add(g.cmul(M4 * D4 * D5, c234), g.cmul(-M5, g.mul(B6, k63)))
    as5 = g.mul(A6, s5)
    bc45 = g.mul(B6, c45)
    M04 = g.add(g.cmul(M5 * D6, as5), g.cmul(-M5 * D6, bc45))

    # M block j,k in {1,2,3}
    Q2 = g.add(A2s, K2s)
    t1 = g.mul(K2, WK)
    t2 = g.mul(A2, WA)
    u12 = g.add(t1, t2)
    M12 = g.sub(g.sub(M11nr, g.cmul(M1, Q2)), u12)
    M22 = g.add(g.add(M11, g.cmul(M23 + M45 - M1, Q2)), g.cmul(-2.0, u12))
    t3 = g.mul(K3, WK2)
    t4 = g.mul(E, WA2)
    u34 = g.add(t3, t4)
    M13 = g.sub(S45, u34)
    Q3 = g.add(K3s, Es)
    M33 = g.add(g.add(S45, g.cmul(-2.0, u34)),
                g.ts2(Q3, M45, "mult", ROTOR, "add"))
    t5 = g.mul(K2, WK2)
    t6 = g.mul(A2, WA2)
    kk = g.mul(K2, K3)
    ae = g.mul(A2, E)
    M23e = g.add(g.sub(g.sub(M13, t5), t6), g.cmul(M45, g.add(kk, ae)))

    # M column 4 (joint 5); M44 is a constant
    P1 = g.sub(g.mul(K6, c234), g.mul(A6, s234))
    P2 = g.sub(g.mul(K2, c234), g.mul(A2, s234))
    P3 = g.sub(g.mul(K3, c234), g.mul(E, s234))
    M14 = g.cmul(M5 * D6, g.mul(c5, P1))
    M24 = g.sub(M14, g.cmul(M5 * D6, g.mul(c5, P2)))
    M34 = g.sub(M14, g.cmul(M5 * D6, g.mul(c5, P3)))
    M44C = M5 * D6 * D6 + ROTOR

    # direction to hand in the rotated frame
    hxr = g.add(g.mul(c1, hx), g.mul(s1, hy))
    hyr = g.sub(g.mul(s1, hx), g.mul(c1, hy))
    dx = g.sub(hxr, A6)
    dy = g.sub(hyr, B6)
    dz = g.sub(g.add(hz, g.C(-D1)), K6)
    n2 = g.add(g.add(g.sq(dx), g.sq(dy)), g.sq(dz))

    # vd = Je^T d
    vd0 = g.sub(g.mul(A6, dy), g.mul(B6, dx))
    vd1 = g.sub(g.mul(K6, dx), g.mul(A6, dz))
    vd2 = g.sub(vd1, g.sub(g.mul(K2, dx), g.mul(A2, dz)))
    vd3 = g.sub(vd1, g.sub(g.mul(K3, dx), g.mul(E, dz)))
    vd4 = g.cmul(D6, g.add(g.add(g.mul(c45, dx), g.mul(s5, dy)),
                           g.mul(s45, dz)))
    vd = [vd0, vd1, vd2, vd3, vd4]

    M = {(0, 0): M00, (0, 1): M01, (0, 2): M02, (0, 3): M03, (0, 4): M04,
         (1, 1): M11, (1, 2): M12, (1, 3): M13, (1, 4): M14,
         (2, 2): M22, (2, 3): M23e, (2, 4): M24,
         (3, 3): M33, (3, 4): M34}

    # Bordered LDL^T on [[M, vd], [vd^T, 0]] (6x6).  No sqrt: the pivot
    # chain (d -> recip -> C/L updates -> d) stays entirely on DVE.  The
    # last pivot d5 = -vd^T M^{-1} vd = -s, so the solve is integrated.
    Mb = dict(M)
    for j in range(5):
        Mb[(j, 5)] = vd[j]
    C = {}   # C[k,j] = L[k,j] * d_j (unnormalized column entries)
    L = {}   # normalized
    r = []
    for jc in range(5):
        if jc == 0:
            dd = Mb[(0, 0)]
        elif jc == 4:
            # M[4,4] is the constant M44C
            dd = g.ts2(g.mul(C[(4, 0)], L[(4, 0)]), -1.0, "mult", M44C, "add")
            for t in range(1, 4):
                dd = g.sub(dd, g.mul(C[(4, t)], L[(4, t)]))
        else:
            dd = Mb[(jc, jc)]
            for t in range(jc):
                dd = g.sub(dd, g.mul(C[(jc, t)], L[(jc, t)]))
        rj = g.recip(dd)
        r.append(rj)
        for kk2 in range(jc + 1, 6):
            a = Mb[(jc, kk2)] if jc == 0 or (jc, kk2) in Mb else None
            a = Mb[(jc, kk2)]
            for t in range(jc):
                a = g.sub(a, g.mul(C[(kk2, t)], L[(jc, t)]))
            C[(kk2, jc)] = a
            L[(kk2, jc)] = g.mul(a, rj)
    # s = -(0 - sum_t C[5,t]*L[5,t]) = sum_t C[5,t]*L[5,t]
    sacc = None
    for t in range(5):
        p = g.mul(C[(5, t)], L[(5, t)])
        sacc = p if sacc is None else g.add(sacc, p)
    cost_neg = g.mul(g.cmul(-1.0, g.recip(sacc)), n2)
    return g, cost_neg


# ----------------------------------------------------------------------------
# numpy evaluation of the DAG (for validation in test.py)
# ----------------------------------------------------------------------------

def eval_numpy(g, root, chans):
    val = {}
    for n in g.nodes:
        if n.op == "const":
            val[n.id] = np.float32(n.c)
        elif n.op == "in":
            val[n.id] = chans[n.c]
        elif n.op == "add":
            val[n.id] = val[n.args[0].id] + val[n.args[1].id]
        elif n.op == "sub":
            val[n.id] = val[n.args[0].id] - val[n.args[1].id]
        elif n.op == "mul":
            val[n.id] = val[n.args[0].id] * val[n.args[1].id]
        elif n.op == "square":
            val[n.id] = val[n.args[0].id] * val[n.args[0].id]
        elif n.op == "cmul":
            val[n.id] = np.float32(n.c) * val[n.args[0].id]
        elif n.op == "cadd":
            val[n.id] = val[n.args[0].id] + np.float32(n.c)
        elif n.op == "sin":
            sc, b = n.c
            val[n.id] = np.sin(np.float32(sc) * val[n.args[0].id] + np.float32(b))
        elif n.op == "ts2":
            s1, op0, s2, op1 = n.c
            v = val[n.args[0].id]
            for s_, o_ in ((s1, op0), (s2, op1)):
                if o_ == "mult":
                    v = v * np.float32(s_)
                else:
                    v = v + np.float32(s_)
            val[n.id] = v
        elif n.op == "sqrt":
            val[n.id] = np.sqrt(val[n.args[0].id])
        elif n.op == "recip":
            val[n.id] = np.float32(1.0) / val[n.args[0].id]
        else:
            raise ValueError(n.op)
        if n.op != "const":
            val[n.id] = val[n.id].astype(np.float32)
    return val[root.id]


def ref_numpy(x):
    """Full-pipeline numpy reference using the DAG; x [B,H,26] -> [B]."""
    B, H, Cc = x.shape
    N = B * H
    flat = x.reshape(N, Cc).astype(np.float32)
    g, root = build_graph()
    chans = {i: flat[:, SRC_COLS[i]] for i in range(8)}
    cn = eval_numpy(g, root, chans)
    return cn.reshape(B, H).sum(axis=1)


# ----------------------------------------------------------------------------
# planning: STT fusion + ETF list scheduling across dve/act/gps
# ----------------------------------------------------------------------------

# pipelined per-[128,128]-op costs (TimelineSim probe)
COST = {
    ("dve", "tt"): 212.0, ("dve", "stt"): 212.0, ("dve", "ts"): 162.0,
    ("dve", "recip"): 204.0, ("dve", "reduce"): 296.0,
    ("act", "any"): 360.0,
    ("gps", "tt"): 440.0, ("gps", "ts"): 360.0,
}
XLAT = 100.0  # cross-engine semaphore latency


def classify(n):
    """Returns options = [(engine, cost), ...]. GPSIMD (Pool) supports only
    tensor_tensor and tensor_scalar; scalar_tensor_tensor is DVE-only."""
    if n.op == "sin" or n.op == "sqrt":
        return [("act", COST[("act", "any")])]
    if n.op == "recip":
        return [("dve", COST[("dve", "recip")])]
    if n.op == "square":
        return [("dve", COST[("dve", "tt")]), ("act", COST[("act", "any")]),
                ("gps", COST[("gps", "tt")])]
    if n.op in ("cadd", "cmul", "ts2"):
        # ts2 with non-(mult,add) pattern can't be an ACT Copy
        actok = True
        if n.op == "ts2" and (n.c[1], n.c[3]) != ("mult", "add"):
            actok = False
        opts = [("dve", COST[("dve", "ts")])]
        if actok:
            opts.append(("act", COST[("act", "any")]))
        opts.append(("gps", COST[("gps", "ts")]))
        return opts
    if n.op in ("add", "sub", "mul"):
        if isinstance(n.c, tuple) and n.c and n.c[0] == "stt_cmul":
            return [("dve", COST[("dve", "stt")])]
        return [("dve", COST[("dve", "tt")]), ("gps", COST[("gps", "tt")])]
    if n.op == "cmul_stt":  # cmul fused with mul/square arg
        return [("dve", COST[("dve", "stt")])]
    raise ValueError(n.op)


def plan(g, root):
    """STT fusion + ETF scheduling. Returns emit list ordered by virtual
    start time, with n.engine set."""
    # reachability + users
    reach = set()
    stack = [root]
    while stack:
        n = stack.pop()
        if n.id in reach:
            continue
        reach.add(n.id)
        stack.extend(n.args)
    for n in g.nodes:
        n.users = []
    order = [n for n in g.nodes if n.id in reach]
    for n in order:
        for a in n.args:
            a.users.append(n)

    # fusion: add/sub(x, cmul(c,y)) -> STT ; cmul(c, mul(x,y)/square(x)) -> STT
    for n in order:
        if n.op in ("add", "sub"):
            for k, a in enumerate(n.args):
                if a.op == "cmul" and len(a.users) == 1 and a.fused_into is None \
                        and a.args[0].fused_into is None \
                        and a.args[0].op != "const":
                    n.c = ("stt_cmul", k, a.c)
                    a.fused_into = n
                    break
        elif n.op == "cmul" and n.fused_into is None:
            a = n.args[0]
            if a.op in ("mul", "square") and len(a.users) == 1 \
                    and a.fused_into is None \
                    and all(aa.fused_into is None for aa in a.args):
                a.fused_into = n

    # effective deps of an emitted node (skipping fused producers)
    def deps(n):
        out = []
        for a in n.args:
            if a.fused_into is n:
                for aa in a.args:
                    if aa.op not in ("const", "in"):
                        out.append(aa)
            elif a.op not in ("const", "in"):
                out.append(a)
        return out

    emit_nodes = [n for n in order
                  if n.op not in ("const", "in") and n.fused_into is None]

    def opts_of(n):
        if n.op == "cmul" and n.args[0].fused_into is n:
            return classify(Expr("cmul_stt"))
        return classify(n)

    # critical-path priority (min cost per node)
    mincost = {n.id: min(c for _, c in opts_of(n)) for n in emit_nodes}
    prio = {}

    def get_prio(n):
        if n.id in prio:
            return prio[n.id]
        p = mincost[n.id] + max(
            (get_prio(u if u.fused_into is None else u.fused_into)
             for u in n.users if (u.fused_into is None or u.fused_into is not n)
             ), default=0.0)
        prio[n.id] = p
        return p

    import sys
    sys.setrecursionlimit(10000)
    for n in emit_nodes:
        n.prio = 0.0
    # compute prios iteratively in reverse topological order (nodes list is topo)
    for n in reversed(emit_nodes):
        best = 0.0
        for u in n.users:
            tgt = u.fused_into if u.fused_into is not None else u
            if tgt is n:
                continue
            if tgt.fused_into is None and tgt.op not in ("const", "in"):
                best = max(best, tgt.prio)
        n.prio = mincost[n.id] + best

    # ---- phase 1: static engine assignment (balance max load) ----
    # Critical-chain nodes keep their fastest engine; the rest greedily go to
    # the engine with the smallest resulting load.
    ndeps = {n.id: 0 for n in emit_nodes}
    dep_lists = {}
    for n in emit_nodes:
        dl = deps(n)
        dep_lists[n.id] = dl
        ndeps[n.id] = len(dl)
    users_emit = {n.id: [] for n in emit_nodes}
    for n in emit_nodes:
        for d in dep_lists[n.id]:
            users_emit[d.id].append(n)

    def run_etf(gamma, win, xlat):
        """ETF with load-penalty engine choice. Returns (makespan, sched:
        list of (n, engine, start, finish))."""
        nd = dict(ndeps)
        ready = [n for n in emit_nodes if nd[n.id] == 0]
        eng_free = {"dve": 0.0, "act": 0.0, "gps": 0.0}
        eload = {"dve": 0.0, "act": 0.0, "gps": 0.0}
        fin = {}
        eng_of = {}
        sched = []
        while ready:
            cands = []
            for n in ready:
                dr_cache = {}
                for e, c in opts_of(n):
                    dr = 0.0
                    for d in dep_lists[n.id]:
                        dr = max(dr, fin[d.id] +
                                 (xlat if eng_of[d.id] != e else 0.0))
                    st = max(eng_free[e], dr)
                    score = st + c + gamma * eload[e]
                    cands.append((score, n.prio, n, e, c, st))
            smin = min(c[0] for c in cands)
            _, _, n, e, c, st = max(
                (cd for cd in cands if cd[0] <= smin + win),
                key=lambda cd: (cd[1], -cd[0]))
            ready.remove(n)
            fin[n.id] = st + c
            eng_of[n.id] = e
            eng_free[e] = st + c
            eload[e] += c
            sched.append((n, e, st, st + c))
            for u in users_emit[n.id]:
                nd[u.id] -= 1
                if nd[u.id] == 0:
                    ready.append(u)
        return max(f for _, _, _, f in sched), sched

    best_ms, best_sched = None, None
    for gamma in (0.0, 0.05, 0.1, 0.2):
        for win in (0.0, 150.0):
            ms, sched = run_etf(gamma, win, XLAT)
            if best_ms is None or ms < best_ms:
                best_ms, best_sched = ms, sched

    load = {"dve": 0.0, "act": 0.0, "gps": 0.0}
    for n, e, st, f in best_sched:
        n.engine = e
        n.start = st
        n.finish = f
        load[e] += f - st

    scheduled = [n for n, _, _, _ in best_sched]
    scheduled.sort(key=lambda n: (n.start, n.finish))
    for i2, n in enumerate(scheduled):
        n.order = i2
    makespan = best_ms
    return scheduled, load, makespan


# ----------------------------------------------------------------------------
# bass emission
# ----------------------------------------------------------------------------

NCORES = 8
B_FULL, H, CH = 2048, 64, 26
N_PER_CORE = B_FULL * H // NCORES          # 16384
P = 128
FD = N_PER_CORE // P                        # 128
NCH = 8


def _build_bass():
    import concourse.bass as bass
    from concourse.bacc import Bacc
    import concourse.mybir as mybir
    from concourse.tile import TileContext

    f32 = mybir.dt.float32
    alu = mybir.AluOpType
    AF = mybir.ActivationFunctionType

    g, root = build_graph()
    emit, load, makespan = plan(g, root)

    nc = Bacc()
    # const APs for non-Copy activation biases
    for cv in (PI / 2,):
        t = nc.alloc_sbuf_tensor(f"constf32-{cv}", [128, 1], f32)
        nc.gpsimd.memset(t.ap(), cv)
        nc.const_aps.aps[(f32, float(cv))] = t.ap()
    nc.all_engine_barrier()
    xs = nc.dram_tensor("xs", (P, NCH * FD), f32, kind="ExternalInput")
    out = nc.dram_tensor("out", (B_FULL // NCORES,), f32, kind="ExternalOutput")

    # liveness for slot allocation
    last_use = {}
    for n in emit:
        for a in n.args:
            if a.order is not None:
                last_use[a.id] = max(last_use.get(a.id, -1), n.order)
            if a.fused_into is n:
                for aa in a.args:
                    if aa.order is not None:
                        last_use[aa.id] = max(last_use.get(aa.id, -1), n.order)
    last_use[root.id] = len(emit) + 10

    with TileContext(nc) as tc:
        with tc.tile_pool(name="vals", bufs=1) as vp:
            # three staged input groups: [q2 q3 q4] [q1 q5] [hx hy hz]
            stA = vp.tile([P, 3 * FD], f32, tag="stA", name="stA")
            stB = vp.tile([P, 2 * FD], f32, tag="stB", name="stB")
            stC = vp.tile([P, 3 * FD], f32, tag="stC", name="stC")
            nc.gpsimd.dma_start(stA[:, :], xs[:, 0:3 * FD])
            nc.gpsimd.dma_start(stB[:, :], xs[:, 3 * FD:5 * FD])
            nc.gpsimd.dma_start(stC[:, :], xs[:, 5 * FD:8 * FD])

            def chan_ap(ch):
                if ch < 3:
                    return stA[:, ch * FD:(ch + 1) * FD]
                if ch < 5:
                    return stB[:, (ch - 3) * FD:(ch - 2) * FD]
                return stC[:, (ch - 5) * FD:(ch - 4) * FD]

            from collections import deque
            free_slots = deque()
            SLACK = 64
            n_slots = [0]
            node_tile = {}

            def ap_of(n):
                if n.op == "in":
                    return chan_ap(n.c)
                return node_tile[n.id][:, :]

            def alloc(n):
                if len(free_slots) > SLACK:
                    sl = free_slots.popleft()
                else:
                    sl = n_slots[0]
                    n_slots[0] += 1
                t = vp.tile([P, FD], f32, tag=f"s{sl}", name=f"v{n.id}", bufs=2)
                n.slot = sl
                node_tile[n.id] = t
                return t

            by_last = {}
            for nid, lu in last_use.items():
                by_last.setdefault(lu, []).append(nid)

            eng = {"dve": nc.vector, "act": nc.scalar, "gps": nc.gpsimd}
            ALU_OF = {"add": alu.add, "sub": alu.subtract, "mul": alu.mult}

            def emit_tt(e, engname, ot, a, b, op):
                e.tensor_tensor(ot, a, b, op)

            for n in emit:
                ot = alloc(n)[:, :]
                e = eng[n.engine]
                en = n.engine
                if n.op == "sin":
                    sc, b = n.c
                    nc.scalar.activation(ot, ap_of(n.args[0]), AF.Sin,
                                         bias=float(b), scale=float(sc))
                elif n.op == "sqrt":
                    nc.scalar.activation(ot, ap_of(n.args[0]), AF.Sqrt)
                elif n.op == "recip":
                    nc.vector.reciprocal_approx_fast(out=ot, in_=ap_of(n.args[0]))
                elif n.op == "square":
                    if en == "act":
                        nc.scalar.activation(ot, ap_of(n.args[0]), AF.Square)
                    else:
                        a = ap_of(n.args[0])
                        emit_tt(e, en, ot, a, a, alu.mult)
                elif n.op == "cadd":
                    if en == "act":
                        nc.scalar.activation(ot, ap_of(n.args[0]), AF.Copy,
                                             bias=float(n.c), scale=1.0)
                    else:
                        e.tensor_scalar_add(ot, ap_of(n.args[0]), float(n.c))
                elif n.op == "ts2":
                    s1, op0, s2, op1 = n.c
                    if en == "act":
                        nc.scalar.activation(ot, ap_of(n.args[0]), AF.Copy,
                                             bias=float(s2), scale=float(s1))
                    else:
                        e.tensor_scalar(ot, ap_of(n.args[0]), float(s1), float(s2),
                                        getattr(alu, op0), getattr(alu, op1))
                elif n.op == "cmul":
                    a = n.args[0]
                    if a.fused_into is n:
                        if a.op == "square":
                            x = yv = a.args[0]
                        else:
                            x, yv = a.args
                        e.scalar_tensor_tensor(ot, ap_of(x), float(n.c),
                                               ap_of(yv), alu.mult, alu.mult)
                    elif en == "act":
                        nc.scalar.activation(ot, ap_of(n.args[0]), AF.Copy,
                                             bias=0.0, scale=float(n.c))
                    else:
                        e.tensor_scalar_mul(ot, ap_of(n.args[0]), float(n.c))
                elif n.op in ("add", "sub"):
                    if isinstance(n.c, tuple) and n.c and n.c[0] == "stt_cmul":
                        _, k, cval = n.c
                        cm = n.args[k]
                        other = n.args[1 - k]
                        x = cm.args[0]
                        if n.op == "add":
                            e.scalar_tensor_tensor(ot, ap_of(x), float(cval),
                                                   ap_of(other), alu.mult, alu.add)
                        else:
                            if k == 1:
                                e.scalar_tensor_tensor(ot, ap_of(x), float(-cval),
                                                       ap_of(other), alu.mult,
                                                       alu.add)
                            else:
                                e.scalar_tensor_tensor(ot, ap_of(x), float(cval),
                                                       ap_of(other), alu.mult,
                                                       alu.subtract)
                    else:
                        emit_tt(e, en, ot, ap_of(n.args[0]), ap_of(n.args[1]),
                                ALU_OF[n.op])
                elif n.op == "mul":
                    emit_tt(e, en, ot, ap_of(n.args[0]), ap_of(n.args[1]),
                            alu.mult)
                else:
                    raise ValueError(n.op)

                for nid in by_last.get(n.order, []):
                    nd = g.nodes[nid]
                    if nd.slot is not None and nd.id != root.id:
                        free_slots.append(nd.slot)
                        nd.slot = None

            # epilogue: per-b sums (64-sample segments)
            osum = vp.tile([P, 2], f32, tag="osum", bufs=2)
            croot = node_tile[root.id]
            nc.vector.tensor_reduce(osum[:, 0:1], croot[:, 0:64],
                                    mybir.AxisListType.X, alu.add)
            nc.vector.tensor_reduce(osum[:, 1:2], croot[:, 64:128],
                                    mybir.AxisListType.X, alu.add)
            nc.gpsimd.dma_start(out.rearrange("(p j) -> p j", p=P), osum[:, :])

    nc.compile()
    return nc, len(emit), load, makespan


_CACHE = {}


def kernel(x, cond, time):
    from concourse.bass_utils import run_bass_kernel_spmd

    if "nc" not in _CACHE:
        nc, n_ops, load, makespan = _build_bass()
        _CACHE["nc"] = nc
        _CACHE["stats"] = (n_ops, load, makespan)
    nc = _CACHE["nc"]

    xf = np.ascontiguousarray(x, dtype=np.float32).reshape(B_FULL * H, CH)
    sel = xf[:, SRC_COLS]                                   # [131072, 8]
    in_maps = []
    for k in range(NCORES):
        shard = sel[k * N_PER_CORE:(k + 1) * N_PER_CORE]    # [16384, 8]
        # [128 p, 128 q, 8 c] -> [128, 8, 128] channel-major free layout
        arr = shard.reshape(P, FD, NCH).transpose(0, 2, 1)
        in_maps.append({"xs": np.ascontiguousarray(arr).reshape(P, NCH * FD)})
    res = run_bass_kernel_spmd(nc, in_maps, core_ids=list(range(NCORES)))
    _CACHE["exec_time_ns"] = res.exec_time_ns
    _CACHE["trace"] = res.instructions_and_trace
    outs = [res.results[k]["out"] for k in range(NCORES)]
    return np.concatenate(outs).astype(np.float32)


if __name__ == "__main__":
    g, root = build_graph()
    emit, load, makespan = plan(g, root)
    from collections import Counter
    print("emitted ops:", len(emit))
    print(Counter((n.engine, n.op) for n in emit))
    print("load est (us):", {k: v / 1000 for k, v in load.items()})
    print("virtual makespan (us):", makespan / 1000)


# revision 23
# speedup vs baseline: 1.0283x; 1.0283x over previous
"""Trainium2 Bass kernel for the UR5e reflected-mass cost function.

Closed-form math (per sample n of 131072 = 2048 b x 64 h):
  The last joint (q6) never affects the output (its Jacobian column is 0),
  and in the q1-rotated "cylindrical" frame every frame origin is
  p_i = (A_i, B_i, C_i) with the z-axes {z0=ez, z1=z2=z3=(0,1,0),
  z4=(s234,0,-c234)}.  All Jacobian columns, the 5x5 mass matrix, and the
  end-effector direction reduce to ~260 scalar ops instead of the naive
  ~670 of the frame-by-frame DH chain.

Implementation: every per-sample scalar is a [128,128] f32 SBUF tile
(16384 samples per core, 8 cores data-parallel over b).  The computation
is a symbolic scalar DAG with CSE + constant folding + STT fusion,
scheduled onto the DVE/ACT/GPSIMD engines with an earliest-finish-time
list scheduler and emitted through the Tile framework.
"""

import math
import numpy as np

# ----------------------------------------------------------------------------
# constants
# ----------------------------------------------------------------------------

PI = math.pi
A2C, A3C = -0.425, -0.3922
D1, D4, D5, D6 = 0.1625, 0.1333, 0.0997, 0.0996
# LINK_MASS[i] sits at frame origin p_{i+1}; link 0 (at p1) never moves.
M1, M2, M3, M4, M5 = 8.058, 2.846, 1.37, 1.3, 0.365
M23 = M2 + M3
M45 = M4 + M5
ROTOR = 0.1
MAGIC = 12582912.0  # 1.5 * 2**23 f32 round-to-int trick

# host channel order handed to the device
# 0:q2 1:q3 2:q4 3:q1 4:q5 5:hx 6:hy 7:hz
SRC_COLS = [7, 8, 9, 6, 10, 19, 20, 21]

# ----------------------------------------------------------------------------
# symbolic scalar DAG
# ----------------------------------------------------------------------------


class Expr:
    __slots__ = ("op", "args", "c", "id", "users", "engine", "fused_into",
                 "slot", "order", "prio", "start", "finish")

    def __init__(self, op, args=(), c=None, i=0):
        self.op = op
        self.args = args
        self.c = c
        self.id = i
        self.users = []
        self.engine = None
        self.fused_into = None
        self.slot = None
        self.order = None
        self.prio = 0.0
        self.start = 0.0
        self.finish = 0.0


class Graph:
    def __init__(self):
        self.nodes = []
        self.cse = {}

    def _mk(self, op, args=(), c=None):
        key = (op, tuple(a.id for a in args), c)
        n = self.cse.get(key)
        if n is None:
            n = Expr(op, args, c, len(self.nodes))
            self.nodes.append(n)
            self.cse[key] = n
        return n

    def C(self, v):
        return self._mk("const", c=float(v))

    def IN(self, ch):
        return self._mk("in", c=ch)

    def add(self, x, y):
        if x.op == "const" and y.op == "const":
            return self.C(x.c + y.c)
        if x.op == "const":
            x, y = y, x
        if y.op == "const":
            if y.c == 0.0:
                return x
            return self._mk("cadd", (x,), y.c)
        a, b = (x, y) if x.id <= y.id else (y, x)
        return self._mk("add", (a, b))

    def sub(self, x, y):
        if x.op == "const" and y.op == "const":
            return self.C(x.c - y.c)
        if y.op == "const":
            if y.c == 0.0:
                return x
            return self._mk("cadd", (x,), -y.c)
        if x.op == "const" and x.c == 0.0:
            return self.cmul(-1.0, y)
        if x is y:
            return self.C(0.0)
        return self._mk("sub", (x, y))

    def cmul(self, c, x):
        c = float(c)
        if x.op == "const":
            return self.C(c * x.c)
        if c == 0.0:
            return self.C(0.0)
        if c == 1.0:
            return x
        if x.op == "cmul":
            return self.cmul(c * x.c, x.args[0])
        return self._mk("cmul", (x,), c)

    def mul(self, x, y):
        if x.op == "const":
            return self.cmul(x.c, y)
        if y.op == "const":
            return self.cmul(y.c, x)
        if x.op == "cmul" and y.op == "cmul":
            return self.cmul(x.c * y.c, self.mul(x.args[0], y.args[0]))
        if x.op == "cmul":
            return self.cmul(x.c, self.mul(x.args[0], y))
        if y.op == "cmul":
            return self.cmul(y.c, self.mul(x, y.args[0]))
        if x is y:
            return self._mk("square", (x,))
        a, b = (x, y) if x.id <= y.id else (y, x)
        return self._mk("mul", (a, b))

    def ts2(self, x, s1, op0, s2, op1):
        return self._mk("ts2", (x,), (float(s1), op0, float(s2), op1))

    def sincos(self, q):
        """(sin q, cos q) sharing one range reduction.
        r0 = q - 2*pi*round(q/2pi) in [-pi, pi]; sin = Sin(r0).
        cos = Sin(r0c + pi/2) where r0c = r0 - 2pi*(r0 >= pi/2), keeping the
        Sin argument in [-pi, pi]."""
        inv2pi = 1.0 / (2.0 * PI)
        t1 = self.ts2(q, inv2pi, "mult", MAGIC, "add")
        k = self._mk("cadd", (t1,), -MAGIC)
        r0 = self.add(self.cmul(-2.0 * PI, k), q)  # fuses to one STT
        s = self._mk("sin", (r0,), (1.0, 0.0))
        ge = self._mk("ts2", (r0,), (PI / 2, "is_ge", 1.0, "mult"))
        r0c = self.add(self.cmul(-2.0 * PI, ge), r0)  # STT
        c = self._mk("sin", (r0c,), (1.0, PI / 2))
        return s, c

    def sqrt_(self, x):
        return self._mk("sqrt", (x,))

    def recip(self, x):
        return self._mk("recip", (x,))

    def sq(self, x):
        return self._mk("square", (x,))


def build_graph():
    """Returns (graph, cost_neg_node). cost_neg = -cost per sample."""
    g = Graph()
    q2, q3, q4, q1, q5 = (g.IN(i) for i in range(5))
    hx, hy, hz = (g.IN(5 + i) for i in range(3))

    q23 = g.add(q2, q3)
    q234 = g.add(q23, q4)
    s1, c1 = g.sincos(q1)
    s2, c2 = g.sincos(q2)
    s23, c23 = g.sincos(q23)
    s234, c234 = g.sincos(q234)
    s5, c5 = g.sincos(q5)

    # cylindrical coordinates (relative: A1 = K1 = 0, K = C - d1)
    A2 = g.cmul(A2C, c2)
    E = g.add(A2, g.cmul(A3C, c23))
    K2 = g.cmul(A2C, s2)
    K3 = g.add(K2, g.cmul(A3C, s23))
    cc = g.mul(c234, s5)
    sc_ = g.mul(s234, s5)
    c45 = g.mul(c234, c5)
    s45 = g.mul(s234, c5)
    A5 = g.add(E, g.cmul(D5, s234))
    A6 = g.sub(A5, g.cmul(D6, cc))
    K5 = g.sub(K3, g.cmul(D5, c234))
    K6 = g.sub(K5, g.cmul(D6, sc_))
    B6 = g.ts2(c5, D6, "mult", D4, "add")   # B6 = d4 + d6*c5

    # squares
    A2s, Es, A5s, A6s = g.sq(A2), g.sq(E), g.sq(A5), g.sq(A6)
    K2s, K3s, K5s, K6s = g.sq(K2), g.sq(K3), g.sq(K5), g.sq(K6)
    B6s = g.sq(B6)

    # weighted square sums (suffix style so S45 comes free)
    SA45 = g.add(g.cmul(M5, A6s), g.cmul(M4, A5s))
    SA = g.add(g.add(SA45, g.cmul(M23, Es)), g.cmul(M1, A2s))
    SK45 = g.add(g.cmul(M5, K6s), g.cmul(M4, K5s))
    SK = g.add(g.add(SK45, g.cmul(M23, K3s)), g.cmul(M1, K2s))
    M11nr = g.add(SA, SK)
    M11 = g.add(M11nr, g.C(ROTOR))
    M00 = g.add(g.add(SA, g.cmul(M5, B6s)), g.C((M3 + M4) * D4 * D4 + ROTOR))
    S45 = g.add(SA45, SK45)

    # weighted linear sums
    WK2 = g.add(g.cmul(M4, K5), g.cmul(M5, K6))
    WK = g.add(g.cmul(M23, K3), WK2)
    WA2 = g.add(g.cmul(M4, A5), g.cmul(M5, A6))
    WA = g.add(g.cmul(M23, E), WA2)

    # M row 0 (joint 1 uses (B, A) plane)
    bk6 = g.mul(B6, K6)
    bk2 = g.mul(B6, K2)
    k63 = g.sub(K6, K3)
    M01 = g.add(g.add(g.cmul(-M3 * D4, K3), g.cmul(-M4 * D4, K5)),
                g.cmul(-M5, bk6))
    M02 = g.add(g.add(M01, g.cmul((M3 + M4) * D4, K2)), g.cmul(M5, bk2))
    M03 = g.add(g.cmul(M4 * D4 * D5, c234), g.cmul(-M5, g.mul(B6, k63)))
    as5 = g.mul(A6, s5)
    bc45 = g.mul(B6, c45)
    M04 = g.add(g.cmul(M5 * D6, as5), g.cmul(-M5 * D6, bc45))

    # M block j,k in {1,2,3}
    Q2 = g.add(A2s, K2s)
    t1 = g.mul(K2, WK)
    t2 = g.mul(A2, WA)
    u12 = g.add(t1, t2)
    M12 = g.sub(g.sub(M11nr, g.cmul(M1, Q2)), u12)
    M22 = g.add(g.add(M11, g.cmul(M23 + M45 - M1, Q2)), g.cmul(-2.0, u12))
    t3 = g.mul(K3, WK2)
    t4 = g.mul(E, WA2)
    u34 = g.add(t3, t4)
    M13 = g.sub(S45, u34)
    Q3 = g.add(K3s, Es)
    M33 = g.add(g.add(S45, g.cmul(-2.0, u34)),
                g.ts2(Q3, M45, "mult", ROTOR, "add"))
    t5 = g.mul(K2, WK2)
    t6 = g.mul(A2, WA2)
    kk = g.mul(K2, K3)
    ae = g.mul(A2, E)
    M23e = g.add(g.sub(g.sub(M13, t5), t6), g.cmul(M45, g.add(kk, ae)))

    # M column 4 (joint 5); M44 is a constant
    P1 = g.sub(g.mul(K6, c234), g.mul(A6, s234))
    P2 = g.sub(g.mul(K2, c234), g.mul(A2, s234))
    P3 = g.sub(g.mul(K3, c234), g.mul(E, s234))
    M14 = g.cmul(M5 * D6, g.mul(c5, P1))
    M24 = g.sub(M14, g.cmul(M5 * D6, g.mul(c5, P2)))
    M34 = g.sub(M14, g.cmul(M5 * D6, g.mul(c5, P3)))
    M44C = M5 * D6 * D6 + ROTOR

    # direction to hand in the rotated frame
    hxr = g.add(g.mul(c1, hx), g.mul(s1, hy))
    hyr = g.sub(g.mul(s1, hx), g.mul(c1, hy))
    dx = g.sub(hxr, A6)
    dy = g.sub(hyr, B6)
    dz = g.sub(g.add(hz, g.C(-D1)), K6)
    n2 = g.add(g.add(g.sq(dx), g.sq(dy)), g.sq(dz))

    # vd = Je^T d
    vd0 = g.sub(g.mul(A6, dy), g.mul(B6, dx))
    vd1 = g.sub(g.mul(K6, dx), g.mul(A6, dz))
    vd2 = g.sub(vd1, g.sub(g.mul(K2, dx), g.mul(A2, dz)))
    vd3 = g.sub(vd1, g.sub(g.mul(K3, dx), g.mul(E, dz)))
    vd4 = g.cmul(D6, g.add(g.add(g.mul(c45, dx), g.mul(s5, dy)),
                           g.mul(s45, dz)))
    vd = [vd0, vd1, vd2, vd3, vd4]

    M = {(0, 0): M00, (0, 1): M01, (0, 2): M02, (0, 3): M03, (0, 4): M04,
         (1, 1): M11, (1, 2): M12, (1, 3): M13, (1, 4): M14,
         (2, 2): M22, (2, 3): M23e, (2, 4): M24,
         (3, 3): M33, (3, 4): M34}

    # Bordered LDL^T on [[M, vd], [vd^T, 0]] (6x6).  No sqrt: the pivot
    # chain (d -> recip -> C/L updates -> d) stays entirely on DVE.  The
    # last pivot d5 = -vd^T M^{-1} vd = -s, so the solve is integrated.
    Mb = dict(M)
    for j in range(5):
        Mb[(j, 5)] = vd[j]
    C = {}   # C[k,j] = L[k,j] * d_j (unnormalized column entries)
    L = {}   # normalized
    r = []
    for jc in range(5):
        if jc == 0:
            dd = Mb[(0, 0)]
        elif jc == 4:
            # M[4,4] is the constant M44C
            dd = g.ts2(g.mul(C[(4, 0)], L[(4, 0)]), -1.0, "mult", M44C, "add")
            for t in range(1, 4):
                dd = g.sub(dd, g.mul(C[(4, t)], L[(4, t)]))
        else:
            dd = Mb[(jc, jc)]
            for t in range(jc):
                dd = g.sub(dd, g.mul(C[(jc, t)], L[(jc, t)]))
        rj = g.recip(dd)
        r.append(rj)
        for kk2 in range(jc + 1, 6):
            a = Mb[(jc, kk2)] if jc == 0 or (jc, kk2) in Mb else None
            a = Mb[(jc, kk2)]
            for t in range(jc):
                a = g.sub(a, g.mul(C[(kk2, t)], L[(jc, t)]))
            C[(kk2, jc)] = a
            L[(kk2, jc)] = g.mul(a, rj)
    # s = -(0 - sum_t C[5,t]*L[5,t]) = sum_t C[5,t]*L[5,t]
    sacc = None
    for t in range(5):
        p = g.mul(C[(5, t)], L[(5, t)])
        sacc = p if sacc is None else g.add(sacc, p)
    cost_neg = g.mul(g.cmul(-1.0, g.recip(sacc)), n2)
    return g, cost_neg


# ----------------------------------------------------------------------------
# numpy evaluation of the DAG (for validation in test.py)
# ----------------------------------------------------------------------------

def eval_numpy(g, root, chans):
    val = {}
    for n in g.nodes:
        if n.op == "const":
            val[n.id] = np.float32(n.c)
        elif n.op == "in":
            val[n.id] = chans[n.c]
        elif n.op == "add":
            val[n.id] = val[n.args[0].id] + val[n.args[1].id]
        elif n.op == "sub":
            val[n.id] = val[n.args[0].id] - val[n.args[1].id]
        elif n.op == "mul":
            val[n.id] = val[n.args[0].id] * val[n.args[1].id]
        elif n.op == "square":
            val[n.id] = val[n.args[0].id] * val[n.args[0].id]
        elif n.op == "cmul":
            val[n.id] = np.float32(n.c) * val[n.args[0].id]
        elif n.op == "cadd":
            val[n.id] = val[n.args[0].id] + np.float32(n.c)
        elif n.op == "sin":
            sc, b = n.c
            val[n.id] = np.sin(np.float32(sc) * val[n.args[0].id] + np.float32(b))
        elif n.op == "ts2":
            s1, op0, s2, op1 = n.c
            v = val[n.args[0].id]
            for s_, o_ in ((s1, op0), (s2, op1)):
                if o_ == "mult":
                    v = v * np.float32(s_)
                elif o_ == "is_ge":
                    v = (v >= np.float32(s_)).astype(np.float32)
                else:
                    v = v + np.float32(s_)
            val[n.id] = v
        elif n.op == "sqrt":
            val[n.id] = np.sqrt(val[n.args[0].id])
        elif n.op == "recip":
            val[n.id] = np.float32(1.0) / val[n.args[0].id]
        else:
            raise ValueError(n.op)
        if n.op != "const":
            val[n.id] = val[n.id].astype(np.float32)
    return val[root.id]


def ref_numpy(x):
    """Full-pipeline numpy reference using the DAG; x [B,H,26] -> [B]."""
    B, H, Cc = x.shape
    N = B * H
    flat = x.reshape(N, Cc).astype(np.float32)
    g, root = build_graph()
    chans = {i: flat[:, SRC_COLS[i]] for i in range(8)}
    cn = eval_numpy(g, root, chans)
    return cn.reshape(B, H).sum(axis=1)


# ----------------------------------------------------------------------------
# planning: STT fusion + ETF list scheduling across dve/act/gps
# ----------------------------------------------------------------------------

# pipelined per-[128,128]-op costs (TimelineSim probe)
COST = {
    ("dve", "tt"): 212.0, ("dve", "stt"): 212.0, ("dve", "ts"): 162.0,
    ("dve", "recip"): 204.0, ("dve", "reduce"): 296.0,
    ("act", "any"): 360.0,
    ("gps", "tt"): 440.0, ("gps", "ts"): 360.0,
}
XLAT = 100.0  # cross-engine semaphore latency


def classify(n):
    """Returns options = [(engine, cost), ...]. GPSIMD (Pool) supports only
    tensor_tensor and tensor_scalar; scalar_tensor_tensor is DVE-only."""
    if n.op == "sin" or n.op == "sqrt":
        return [("act", COST[("act", "any")])]
    if n.op == "recip":
        return [("dve", COST[("dve", "recip")])]
    if n.op == "square":
        return [("dve", COST[("dve", "tt")]), ("act", COST[("act", "any")]),
                ("gps", COST[("gps", "tt")])]
    if n.op in ("cadd", "cmul", "ts2"):
        # ts2 with non-(mult,add) pattern can't be an ACT Copy
        actok = True
        if n.op == "ts2" and (n.c[1], n.c[3]) != ("mult", "add"):
            actok = False
        opts = [("dve", COST[("dve", "ts")])]
        if actok:
            opts.append(("act", COST[("act", "any")]))
        opts.append(("gps", COST[("gps", "ts")]))
        return opts
    if n.op in ("add", "sub", "mul"):
        if isinstance(n.c, tuple) and n.c and n.c[0] == "stt_cmul":
            return [("dve", COST[("dve", "stt")])]
        return [("dve", COST[("dve", "tt")]), ("gps", COST[("gps", "tt")])]
    if n.op == "cmul_stt":  # cmul fused with mul/square arg
        return [("dve", COST[("dve", "stt")])]
    raise ValueError(n.op)


def plan(g, root):
    """STT fusion + ETF scheduling. Returns emit list ordered by virtual
    start time, with n.engine set."""
    # reachability + users
    reach = set()
    stack = [root]
    while stack:
        n = stack.pop()
        if n.id in reach:
            continue
        reach.add(n.id)
        stack.extend(n.args)
    for n in g.nodes:
        n.users = []
    order = [n for n in g.nodes if n.id in reach]
    for n in order:
        for a in n.args:
            a.users.append(n)

    # fusion: add/sub(x, cmul(c,y)) -> STT ; cmul(c, mul(x,y)/square(x)) -> STT
    for n in order:
        if n.op in ("add", "sub"):
            for k, a in enumerate(n.args):
                if a.op == "cmul" and len(a.users) == 1 and a.fused_into is None \
                        and a.args[0].fused_into is None \
                        and a.args[0].op != "const":
                    n.c = ("stt_cmul", k, a.c)
                    a.fused_into = n
                    break
        elif n.op == "cmul" and n.fused_into is None:
            a = n.args[0]
            if a.op in ("mul", "square") and len(a.users) == 1 \
                    and a.fused_into is None \
                    and all(aa.fused_into is None for aa in a.args):
                a.fused_into = n

    # effective deps of an emitted node (skipping fused producers)
    def deps(n):
        out = []
        for a in n.args:
            if a.fused_into is n:
                for aa in a.args:
                    if aa.op not in ("const", "in"):
                        out.append(aa)
            elif a.op not in ("const", "in"):
                out.append(a)
        return out

    emit_nodes = [n for n in order
                  if n.op not in ("const", "in") and n.fused_into is None]

    def opts_of(n):
        if n.op == "cmul" and n.args[0].fused_into is n:
            return classify(Expr("cmul_stt"))
        return classify(n)

    # critical-path priority (min cost per node)
    mincost = {n.id: min(c for _, c in opts_of(n)) for n in emit_nodes}
    prio = {}

    def get_prio(n):
        if n.id in prio:
            return prio[n.id]
        p = mincost[n.id] + max(
            (get_prio(u if u.fused_into is None else u.fused_into)
             for u in n.users if (u.fused_into is None or u.fused_into is not n)
             ), default=0.0)
        prio[n.id] = p
        return p

    import sys
    sys.setrecursionlimit(10000)
    for n in emit_nodes:
        n.prio = 0.0
    # compute prios iteratively in reverse topological order (nodes list is topo)
    for n in reversed(emit_nodes):
        best = 0.0
        for u in n.users:
            tgt = u.fused_into if u.fused_into is not None else u
            if tgt is n:
                continue
            if tgt.fused_into is None and tgt.op not in ("const", "in"):
                best = max(best, tgt.prio)
        n.prio = mincost[n.id] + best

    # ---- phase 1: static engine assignment (balance max load) ----
    # Critical-chain nodes keep their fastest engine; the rest greedily go to
    # the engine with the smallest resulting load.
    ndeps = {n.id: 0 for n in emit_nodes}
    dep_lists = {}
    for n in emit_nodes:
        dl = deps(n)
        dep_lists[n.id] = dl
        ndeps[n.id] = len(dl)
    users_emit = {n.id: [] for n in emit_nodes}
    for n in emit_nodes:
        for d in dep_lists[n.id]:
            users_emit[d.id].append(n)

    def run_etf(gamma, win, xlat):
        """ETF with load-penalty engine choice. Returns (makespan, sched:
        list of (n, engine, start, finish))."""
        nd = dict(ndeps)
        ready = [n for n in emit_nodes if nd[n.id] == 0]
        eng_free = {"dve": 0.0, "act": 0.0, "gps": 0.0}
        eload = {"dve": 0.0, "act": 0.0, "gps": 0.0}
        fin = {}
        eng_of = {}
        sched = []
        while ready:
            cands = []
            for n in ready:
                dr_cache = {}
                for e, c in opts_of(n):
                    dr = 0.0
                    for d in dep_lists[n.id]:
                        dr = max(dr, fin[d.id] +
                                 (xlat if eng_of[d.id] != e else 0.0))
                    st = max(eng_free[e], dr)
                    score = st + c + gamma * eload[e]
                    cands.append((score, n.prio, n, e, c, st))
            smin = min(c[0] for c in cands)
            _, _, n, e, c, st = max(
                (cd for cd in cands if cd[0] <= smin + win),
                key=lambda cd: (cd[1], -cd[0]))
            ready.remove(n)
            fin[n.id] = st + c
            eng_of[n.id] = e
            eng_free[e] = st + c
            eload[e] += c
            sched.append((n, e, st, st + c))
            for u in users_emit[n.id]:
                nd[u.id] -= 1
                if nd[u.id] == 0:
                    ready.append(u)
        return max(f for _, _, _, f in sched), sched

    best_ms, best_sched = None, None
    for gamma in (0.0, 0.05, 0.1, 0.2):
        for win in (0.0, 150.0):
            ms, sched = run_etf(gamma, win, XLAT)
            if best_ms is None or ms < best_ms:
                best_ms, best_sched = ms, sched

    load = {"dve": 0.0, "act": 0.0, "gps": 0.0}
    for n, e, st, f in best_sched:
        n.engine = e
        n.start = st
        n.finish = f
        load[e] += f - st

    scheduled = [n for n, _, _, _ in best_sched]
    scheduled.sort(key=lambda n: (n.start, n.finish))
    for i2, n in enumerate(scheduled):
        n.order = i2
    makespan = best_ms
    return scheduled, load, makespan


# ----------------------------------------------------------------------------
# bass emission
# ----------------------------------------------------------------------------

NCORES = 8
B_FULL, H, CH = 2048, 64, 26
N_PER_CORE = B_FULL * H // NCORES          # 16384
P = 128
FD = N_PER_CORE // P                        # 128
NCH = 8


def _build_bass():
    import concourse.bass as bass
    from concourse.bacc import Bacc
    import concourse.mybir as mybir
    from concourse.tile import TileContext

    f32 = mybir.dt.float32
    alu = mybir.AluOpType
    AF = mybir.ActivationFunctionType

    g, root = build_graph()
    emit, load, makespan = plan(g, root)

    nc = Bacc()
    xs = nc.dram_tensor("xs", (P, NCH * FD), f32, kind="ExternalInput")
    out = nc.dram_tensor("out", (B_FULL // NCORES,), f32, kind="ExternalOutput")

    # liveness for slot allocation
    last_use = {}
    for n in emit:
        for a in n.args:
            if a.order is not None:
                last_use[a.id] = max(last_use.get(a.id, -1), n.order)
            if a.fused_into is n:
                for aa in a.args:
                    if aa.order is not None:
                        last_use[aa.id] = max(last_use.get(aa.id, -1), n.order)
    last_use[root.id] = len(emit) + 10

    with TileContext(nc) as tc:
        with tc.tile_pool(name="vals", bufs=1) as vp:
            # three staged input groups: [q2 q3 q4] [q1 q5] [hx hy hz],
            # issued at t=0 on three different HWDGE-capable engines so the
            # fixed DGE latencies overlap; transfers serialize on the DMA bus
            # in issue order (q2/q3/q4 first — head of the trig chain).
            stA = vp.tile([P, 3 * FD], f32, tag="stA", name="stA")
            stB = vp.tile([P, 2 * FD], f32, tag="stB", name="stB")
            stC = vp.tile([P, 3 * FD], f32, tag="stC", name="stC")
            nc.sync.dma_start(stA[:, :], xs[:, 0:3 * FD])
            nc.scalar.dma_start(stB[:, :], xs[:, 3 * FD:5 * FD])
            nc.gpsimd.dma_start(stC[:, :], xs[:, 5 * FD:8 * FD])
            # const APs for non-Copy activation biases (registered after the
            # DMAs so they don't delay them; barrier orders memset vs readers)
            for cv in (PI / 2,):
                t = nc.alloc_sbuf_tensor(f"constf32-{cv}", [128, 1], f32)
                nc.gpsimd.memset(t.ap(), cv)
                nc.const_aps.aps[(f32, float(cv))] = t.ap()
            nc.all_engine_barrier()

            def chan_ap(ch):
                if ch < 3:
                    return stA[:, ch * FD:(ch + 1) * FD]
                if ch < 5:
                    return stB[:, (ch - 3) * FD:(ch - 2) * FD]
                return stC[:, (ch - 5) * FD:(ch - 4) * FD]

            from collections import deque
            free_slots = deque()
            SLACK = 64
            n_slots = [0]
            node_tile = {}

            def ap_of(n):
                if n.op == "in":
                    return chan_ap(n.c)
                return node_tile[n.id][:, :]

            def alloc(n):
                if len(free_slots) > SLACK:
                    sl = free_slots.popleft()
                else:
                    sl = n_slots[0]
                    n_slots[0] += 1
                t = vp.tile([P, FD], f32, tag=f"s{sl}", name=f"v{n.id}", bufs=2)
                n.slot = sl
                node_tile[n.id] = t
                return t

            by_last = {}
            for nid, lu in last_use.items():
                by_last.setdefault(lu, []).append(nid)

            eng = {"dve": nc.vector, "act": nc.scalar, "gps": nc.gpsimd}
            ALU_OF = {"add": alu.add, "sub": alu.subtract, "mul": alu.mult}

            def emit_tt(e, engname, ot, a, b, op):
                e.tensor_tensor(ot, a, b, op)

            for n in emit:
                ot = alloc(n)[:, :]
                e = eng[n.engine]
                en = n.engine
                if n.op == "sin":
                    sc, b = n.c
                    nc.scalar.activation(ot, ap_of(n.args[0]), AF.Sin,
                                         bias=float(b), scale=float(sc))
                elif n.op == "sqrt":
                    nc.scalar.activation(ot, ap_of(n.args[0]), AF.Sqrt)
                elif n.op == "recip":
                    nc.vector.reciprocal_approx_fast(out=ot, in_=ap_of(n.args[0]))
                elif n.op == "square":
                    if en == "act":
                        nc.scalar.activation(ot, ap_of(n.args[0]), AF.Square)
                    else:
                        a = ap_of(n.args[0])
                        emit_tt(e, en, ot, a, a, alu.mult)
                elif n.op == "cadd":
                    if en == "act":
                        nc.scalar.activation(ot, ap_of(n.args[0]), AF.Copy,
                                             bias=float(n.c), scale=1.0)
                    else:
                        e.tensor_scalar_add(ot, ap_of(n.args[0]), float(n.c))
                elif n.op == "ts2":
                    s1, op0, s2, op1 = n.c
                    if en == "act":
                        nc.scalar.activation(ot, ap_of(n.args[0]), AF.Copy,
                                             bias=float(s2), scale=float(s1))
                    else:
                        e.tensor_scalar(ot, ap_of(n.args[0]), float(s1), float(s2),
                                        getattr(alu, op0), getattr(alu, op1))
                elif n.op == "cmul":
                    a = n.args[0]
                    if a.fused_into is n:
                        if a.op == "square":
                            x = yv = a.args[0]
                        else:
                            x, yv = a.args
                        e.scalar_tensor_tensor(ot, ap_of(x), float(n.c),
                                               ap_of(yv), alu.mult, alu.mult)
                    elif en == "act":
                        nc.scalar.activation(ot, ap_of(n.args[0]), AF.Copy,
                                             bias=0.0, scale=float(n.c))
                    else:
                        e.tensor_scalar_mul(ot, ap_of(n.args[0]), float(n.c))
                elif n.op in ("add", "sub"):
                    if isinstance(n.c, tuple) and n.c and n.c[0] == "stt_cmul":
                        _, k, cval = n.c
                        cm = n.args[k]
                        other = n.args[1 - k]
                        x = cm.args[0]
                        if n.op == "add":
                            e.scalar_tensor_tensor(ot, ap_of(x), float(cval),
                                                   ap_of(other), alu.mult, alu.add)
                        else:
                            if k == 1:
                                e.scalar_tensor_tensor(ot, ap_of(x), float(-cval),
                                                       ap_of(other), alu.mult,
                                                       alu.add)
                            else:
                                e.scalar_tensor_tensor(ot, ap_of(x), float(cval),
                                                       ap_of(other), alu.mult,
                                                       alu.subtract)
                    else:
                        emit_tt(e, en, ot, ap_of(n.args[0]), ap_of(n.args[1]),
                                ALU_OF[n.op])
                elif n.op == "mul":
                    emit_tt(e, en, ot, ap_of(n.args[0]), ap_of(n.args[1]),
                            alu.mult)
                else:
                    raise ValueError(n.op)

                for nid in by_last.get(n.order, []):
                    nd = g.nodes[nid]
                    if nd.slot is not None and nd.id != root.id:
                        free_slots.append(nd.slot)
                        nd.slot = None

            # epilogue: per-b sums (64-sample segments)
            osum = vp.tile([P, 2], f32, tag="osum", bufs=2)
            croot = node_tile[root.id]
            nc.vector.tensor_reduce(osum[:, 0:1], croot[:, 0:64],
                                    mybir.AxisListType.X, alu.add)
            nc.vector.tensor_reduce(osum[:, 1:2], croot[:, 64:128],
                                    mybir.AxisListType.X, alu.add)
            nc.gpsimd.dma_start(out.rearrange("(p j) -> p j", p=P), osum[:, :])

    nc.compile()
    return nc, len(emit), load, makespan


_CACHE = {}


def kernel(x, cond, time):
    from concourse.bass_utils import run_bass_kernel_spmd

    if "nc" not in _CACHE:
        nc, n_ops, load, makespan = _build_bass()
        _CACHE["nc"] = nc
        _CACHE["stats"] = (n_ops, load, makespan)
    nc = _CACHE["nc"]

    xf = np.ascontiguousarray(x, dtype=np.float32).reshape(B_FULL * H, CH)
    sel = xf[:, SRC_COLS]                                   # [131072, 8]
    in_maps = []
    for k in range(NCORES):
        shard = sel[k * N_PER_CORE:(k + 1) * N_PER_CORE]    # [16384, 8]
        # [128 p, 128 q, 8 c] -> [128, 8, 128] channel-major free layout
        arr = shard.reshape(P, FD, NCH).transpose(0, 2, 1)
        in_maps.append({"xs": np.ascontiguousarray(arr).reshape(P, NCH * FD)})
    res = run_bass_kernel_spmd(nc, in_maps, core_ids=list(range(NCORES)))
    _CACHE["exec_time_ns"] = res.exec_time_ns
    _CACHE["trace"] = res.instructions_and_trace
    outs = [res.results[k]["out"] for k in range(NCORES)]
    return np.concatenate(outs).astype(np.float32)


if __name__ == "__main__":
    g, root = build_graph()
    emit, load, makespan = plan(g, root)
    from collections import Counter
    print("emitted ops:", len(emit))
    print(Counter((n.engine, n.op) for n in emit))
    print("load est (us):", {k: v / 1000 for k, v in load.items()})
    print("virtual makespan (us):", makespan / 1000)


# revision 24
# speedup vs baseline: 1.0845x; 1.0547x over previous
"""Trainium2 Bass kernel for the UR5e reflected-mass cost function.

Closed-form math (per sample n of 131072 = 2048 b x 64 h):
  The last joint (q6) never affects the output (its Jacobian column is 0),
  and in the q1-rotated "cylindrical" frame every frame origin is
  p_i = (A_i, B_i, C_i) with the z-axes {z0=ez, z1=z2=z3=(0,1,0),
  z4=(s234,0,-c234)}.  All Jacobian columns, the 5x5 mass matrix, and the
  end-effector direction reduce to ~260 scalar ops instead of the naive
  ~670 of the frame-by-frame DH chain.

Implementation: every per-sample scalar is a [128,128] f32 SBUF tile
(16384 samples per core, 8 cores data-parallel over b).  The computation
is a symbolic scalar DAG with CSE + constant folding + STT fusion,
scheduled onto the DVE/ACT/GPSIMD engines with an earliest-finish-time
list scheduler and emitted through the Tile framework.
"""

import math
import numpy as np

# ----------------------------------------------------------------------------
# constants
# ----------------------------------------------------------------------------

PI = math.pi
A2C, A3C = -0.425, -0.3922
D1, D4, D5, D6 = 0.1625, 0.1333, 0.0997, 0.0996
# LINK_MASS[i] sits at frame origin p_{i+1}; link 0 (at p1) never moves.
M1, M2, M3, M4, M5 = 8.058, 2.846, 1.37, 1.3, 0.365
M23 = M2 + M3
M45 = M4 + M5
ROTOR = 0.1
MAGIC = 12582912.0  # 1.5 * 2**23 f32 round-to-int trick

# host channel order handed to the device
# 0:q2 1:q3 2:q4 3:q1 4:q5 5:hx 6:hy 7:hz
SRC_COLS = [7, 8, 9, 6, 10, 19, 20, 21]

# ----------------------------------------------------------------------------
# symbolic scalar DAG
# ----------------------------------------------------------------------------


class Expr:
    __slots__ = ("op", "args", "c", "id", "users", "engine", "fused_into",
                 "slot", "order", "prio", "start", "finish")

    def __init__(self, op, args=(), c=None, i=0):
        self.op = op
        self.args = args
        self.c = c
        self.id = i
        self.users = []
        self.engine = None
        self.fused_into = None
        self.slot = None
        self.order = None
        self.prio = 0.0
        self.start = 0.0
        self.finish = 0.0


class Graph:
    def __init__(self):
        self.nodes = []
        self.cse = {}

    def _mk(self, op, args=(), c=None):
        key = (op, tuple(a.id for a in args), c)
        n = self.cse.get(key)
        if n is None:
            n = Expr(op, args, c, len(self.nodes))
            self.nodes.append(n)
            self.cse[key] = n
        return n

    def C(self, v):
        return self._mk("const", c=float(v))

    def IN(self, ch):
        return self._mk("in", c=ch)

    def add(self, x, y):
        if x.op == "const" and y.op == "const":
            return self.C(x.c + y.c)
        if x.op == "const":
            x, y = y, x
        if y.op == "const":
            if y.c == 0.0:
                return x
            return self._mk("cadd", (x,), y.c)
        a, b = (x, y) if x.id <= y.id else (y, x)
        return self._mk("add", (a, b))

    def sub(self, x, y):
        if x.op == "const" and y.op == "const":
            return self.C(x.c - y.c)
        if y.op == "const":
            if y.c == 0.0:
                return x
            return self._mk("cadd", (x,), -y.c)
        if x.op == "const" and x.c == 0.0:
            return self.cmul(-1.0, y)
        if x is y:
            return self.C(0.0)
        return self._mk("sub", (x, y))

    def cmul(self, c, x):
        c = float(c)
        if x.op == "const":
            return self.C(c * x.c)
        if c == 0.0:
            return self.C(0.0)
        if c == 1.0:
            return x
        if x.op == "cmul":
            return self.cmul(c * x.c, x.args[0])
        return self._mk("cmul", (x,), c)

    def mul(self, x, y):
        if x.op == "const":
            return self.cmul(x.c, y)
        if y.op == "const":
            return self.cmul(y.c, x)
        if x.op == "cmul" and y.op == "cmul":
            return self.cmul(x.c * y.c, self.mul(x.args[0], y.args[0]))
        if x.op == "cmul":
            return self.cmul(x.c, self.mul(x.args[0], y))
        if y.op == "cmul":
            return self.cmul(y.c, self.mul(x, y.args[0]))
        if x is y:
            return self._mk("square", (x,))
        a, b = (x, y) if x.id <= y.id else (y, x)
        return self._mk("mul", (a, b))

    def ts2(self, x, s1, op0, s2, op1):
        return self._mk("ts2", (x,), (float(s1), op0, float(s2), op1))

    def sincos(self, q):
        """(sin q, cos q) sharing one range reduction.
        r0 = q - 2*pi*round(q/2pi) in [-pi, pi]; sin = Sin(r0).
        cos = Sin(r0c + pi/2) where r0c = r0 - 2pi*(r0 >= pi/2), keeping the
        Sin argument in [-pi, pi]."""
        inv2pi = 1.0 / (2.0 * PI)
        t1 = self.ts2(q, inv2pi, "mult", MAGIC, "add")
        k = self._mk("cadd", (t1,), -MAGIC)
        r0 = self.add(self.cmul(-2.0 * PI, k), q)  # fuses to one STT
        s = self._mk("sin", (r0,), (1.0, 0.0))
        ge = self._mk("ts2", (r0,), (PI / 2, "is_ge", 1.0, "mult"))
        r0c = self.add(self.cmul(-2.0 * PI, ge), r0)  # STT
        c = self._mk("sin", (r0c,), (1.0, PI / 2))
        return s, c

    def sqrt_(self, x):
        return self._mk("sqrt", (x,))

    def recip(self, x):
        return self._mk("recip", (x,))

    def sq(self, x):
        return self._mk("square", (x,))


def build_graph():
    """Returns (graph, cost_neg_node). cost_neg = -cost per sample."""
    g = Graph()
    q2, q3, q4, q1, q5 = (g.IN(i) for i in range(5))
    hx, hy, hz = (g.IN(5 + i) for i in range(3))

    q23 = g.add(q2, q3)
    q234 = g.add(q23, q4)
    s1, c1 = g.sincos(q1)
    s2, c2 = g.sincos(q2)
    s23, c23 = g.sincos(q23)
    s234, c234 = g.sincos(q234)
    s5, c5 = g.sincos(q5)

    # cylindrical coordinates (relative: A1 = K1 = 0, K = C - d1)
    A2 = g.cmul(A2C, c2)
    E = g.add(A2, g.cmul(A3C, c23))
    K2 = g.cmul(A2C, s2)
    K3 = g.add(K2, g.cmul(A3C, s23))
    cc = g.mul(c234, s5)
    sc_ = g.mul(s234, s5)
    c45 = g.mul(c234, c5)
    s45 = g.mul(s234, c5)
    A5 = g.add(E, g.cmul(D5, s234))
    A6 = g.sub(A5, g.cmul(D6, cc))
    K5 = g.sub(K3, g.cmul(D5, c234))
    K6 = g.sub(K5, g.cmul(D6, sc_))
    B6 = g.ts2(c5, D6, "mult", D4, "add")   # B6 = d4 + d6*c5

    # squares
    A2s, Es, A5s, A6s = g.sq(A2), g.sq(E), g.sq(A5), g.sq(A6)
    K2s, K3s, K5s, K6s = g.sq(K2), g.sq(K3), g.sq(K5), g.sq(K6)
    B6s = g.sq(B6)

    # weighted square sums (suffix style so S45 comes free)
    SA45 = g.add(g.cmul(M5, A6s), g.cmul(M4, A5s))
    SA = g.add(g.add(SA45, g.cmul(M23, Es)), g.cmul(M1, A2s))
    SK45 = g.add(g.cmul(M5, K6s), g.cmul(M4, K5s))
    SK = g.add(g.add(SK45, g.cmul(M23, K3s)), g.cmul(M1, K2s))
    M11nr = g.add(SA, SK)
    M11 = g.add(M11nr, g.C(ROTOR))
    M00 = g.add(g.add(SA, g.cmul(M5, B6s)), g.C((M3 + M4) * D4 * D4 + ROTOR))
    S45 = g.add(SA45, SK45)

    # weighted linear sums
    WK2 = g.add(g.cmul(M4, K5), g.cmul(M5, K6))
    WK = g.add(g.cmul(M23, K3), WK2)
    WA2 = g.add(g.cmul(M4, A5), g.cmul(M5, A6))
    WA = g.add(g.cmul(M23, E), WA2)

    # M row 0 (joint 1 uses (B, A) plane)
    bk6 = g.mul(B6, K6)
    bk2 = g.mul(B6, K2)
    k63 = g.sub(K6, K3)
    M01 = g.add(g.add(g.cmul(-M3 * D4, K3), g.cmul(-M4 * D4, K5)),
                g.cmul(-M5, bk6))
    M02 = g.add(g.add(M01, g.cmul((M3 + M4) * D4, K2)), g.cmul(M5, bk2))
    M03 = g.add(g.cmul(M4 * D4 * D5, c234), g.cmul(-M5, g.mul(B6, k63)))
    as5 = g.mul(A6, s5)
    bc45 = g.mul(B6, c45)
    M04 = g.add(g.cmul(M5 * D6, as5), g.cmul(-M5 * D6, bc45))

    # M block j,k in {1,2,3}
    Q2 = g.add(A2s, K2s)
    t1 = g.mul(K2, WK)
    t2 = g.mul(A2, WA)
    u12 = g.add(t1, t2)
    M12 = g.sub(g.sub(M11nr, g.cmul(M1, Q2)), u12)
    M22 = g.add(g.add(M11, g.cmul(M23 + M45 - M1, Q2)), g.cmul(-2.0, u12))
    t3 = g.mul(K3, WK2)
    t4 = g.mul(E, WA2)
    u34 = g.add(t3, t4)
    M13 = g.sub(S45, u34)
    Q3 = g.add(K3s, Es)
    M33 = g.add(g.add(S45, g.cmul(-2.0, u34)),
                g.ts2(Q3, M45, "mult", ROTOR, "add"))
    t5 = g.mul(K2, WK2)
    t6 = g.mul(A2, WA2)
    kk = g.mul(K2, K3)
    ae = g.mul(A2, E)
    M23e = g.add(g.sub(g.sub(M13, t5), t6), g.cmul(M45, g.add(kk, ae)))

    # M column 4 (joint 5); M44 is a constant
    P1 = g.sub(g.mul(K6, c234), g.mul(A6, s234))
    P2 = g.sub(g.mul(K2, c234), g.mul(A2, s234))
    P3 = g.sub(g.mul(K3, c234), g.mul(E, s234))
    M14 = g.cmul(M5 * D6, g.mul(c5, P1))
    M24 = g.sub(M14, g.cmul(M5 * D6, g.mul(c5, P2)))
    M34 = g.sub(M14, g.cmul(M5 * D6, g.mul(c5, P3)))
    M44C = M5 * D6 * D6 + ROTOR

    # direction to hand in the rotated frame
    hxr = g.add(g.mul(c1, hx), g.mul(s1, hy))
    hyr = g.sub(g.mul(s1, hx), g.mul(c1, hy))
    dx = g.sub(hxr, A6)
    dy = g.sub(hyr, B6)
    dz = g.sub(g.add(hz, g.C(-D1)), K6)
    n2 = g.add(g.add(g.sq(dx), g.sq(dy)), g.sq(dz))

    # vd = Je^T d
    vd0 = g.sub(g.mul(A6, dy), g.mul(B6, dx))
    vd1 = g.sub(g.mul(K6, dx), g.mul(A6, dz))
    vd2 = g.sub(vd1, g.sub(g.mul(K2, dx), g.mul(A2, dz)))
    vd3 = g.sub(vd1, g.sub(g.mul(K3, dx), g.mul(E, dz)))
    vd4 = g.cmul(D6, g.add(g.add(g.mul(c45, dx), g.mul(s5, dy)),
                           g.mul(s45, dz)))
    vd = [vd0, vd1, vd2, vd3, vd4]

    M = {(0, 0): M00, (0, 1): M01, (0, 2): M02, (0, 3): M03, (0, 4): M04,
         (1, 1): M11, (1, 2): M12, (1, 3): M13, (1, 4): M14,
         (2, 2): M22, (2, 3): M23e, (2, 4): M24,
         (3, 3): M33, (3, 4): M34}

    # Bordered LDL^T on [[M, vd], [vd^T, 0]] (6x6).  No sqrt: the pivot
    # chain (d -> recip -> C/L updates -> d) stays entirely on DVE.  The
    # last pivot d5 = -vd^T M^{-1} vd = -s, so the solve is integrated.
    Mb = dict(M)
    for j in range(5):
        Mb[(j, 5)] = vd[j]
    C = {}   # C[k,j] = L[k,j] * d_j (unnormalized column entries)
    L = {}   # normalized
    r = []
    for jc in range(5):
        if jc == 0:
            dd = Mb[(0, 0)]
        elif jc == 4:
            # M[4,4] is the constant M44C
            dd = g.ts2(g.mul(C[(4, 0)], L[(4, 0)]), -1.0, "mult", M44C, "add")
            for t in range(1, 4):
                dd = g.sub(dd, g.mul(C[(4, t)], L[(4, t)]))
        else:
            dd = Mb[(jc, jc)]
            for t in range(jc):
                dd = g.sub(dd, g.mul(C[(jc, t)], L[(jc, t)]))
        rj = g.recip(dd)
        r.append(rj)
        for kk2 in range(jc + 1, 6):
            a = Mb[(jc, kk2)] if jc == 0 or (jc, kk2) in Mb else None
            a = Mb[(jc, kk2)]
            for t in range(jc):
                a = g.sub(a, g.mul(C[(kk2, t)], L[(jc, t)]))
            C[(kk2, jc)] = a
            L[(kk2, jc)] = g.mul(a, rj)
    # s = -(0 - sum_t C[5,t]*L[5,t]) = sum_t C[5,t]*L[5,t]
    sacc = None
    for t in range(5):
        p = g.mul(C[(5, t)], L[(5, t)])
        sacc = p if sacc is None else g.add(sacc, p)
    cost_neg = g.mul(g.cmul(-1.0, g.recip(sacc)), n2)
    return g, cost_neg


# ----------------------------------------------------------------------------
# numpy evaluation of the DAG (for validation in test.py)
# ----------------------------------------------------------------------------

def eval_numpy(g, root, chans):
    val = {}
    for n in g.nodes:
        if n.op == "const":
            val[n.id] = np.float32(n.c)
        elif n.op == "in":
            val[n.id] = chans[n.c]
        elif n.op == "add":
            val[n.id] = val[n.args[0].id] + val[n.args[1].id]
        elif n.op == "sub":
            val[n.id] = val[n.args[0].id] - val[n.args[1].id]
        elif n.op == "mul":
            val[n.id] = val[n.args[0].id] * val[n.args[1].id]
        elif n.op == "square":
            val[n.id] = val[n.args[0].id] * val[n.args[0].id]
        elif n.op == "cmul":
            val[n.id] = np.float32(n.c) * val[n.args[0].id]
        elif n.op == "cadd":
            val[n.id] = val[n.args[0].id] + np.float32(n.c)
        elif n.op == "sin":
            sc, b = n.c
            val[n.id] = np.sin(np.float32(sc) * val[n.args[0].id] + np.float32(b))
        elif n.op == "ts2":
            s1, op0, s2, op1 = n.c
            v = val[n.args[0].id]
            for s_, o_ in ((s1, op0), (s2, op1)):
                if o_ == "mult":
                    v = v * np.float32(s_)
                elif o_ == "is_ge":
                    v = (v >= np.float32(s_)).astype(np.float32)
                else:
                    v = v + np.float32(s_)
            val[n.id] = v
        elif n.op == "sqrt":
            val[n.id] = np.sqrt(val[n.args[0].id])
        elif n.op == "recip":
            val[n.id] = np.float32(1.0) / val[n.args[0].id]
        else:
            raise ValueError(n.op)
        if n.op != "const":
            val[n.id] = val[n.id].astype(np.float32)
    return val[root.id]


def ref_numpy(x):
    """Full-pipeline numpy reference using the DAG; x [B,H,26] -> [B]."""
    B, H, Cc = x.shape
    N = B * H
    flat = x.reshape(N, Cc).astype(np.float32)
    g, root = build_graph()
    chans = {i: flat[:, SRC_COLS[i]] for i in range(8)}
    cn = eval_numpy(g, root, chans)
    return cn.reshape(B, H).sum(axis=1)


# ----------------------------------------------------------------------------
# planning: STT fusion + ETF list scheduling across dve/act/gps
# ----------------------------------------------------------------------------

# pipelined per-[128,128]-op costs (TimelineSim probe)
COST = {
    ("dve", "tt"): 212.0, ("dve", "stt"): 212.0, ("dve", "ts"): 162.0,
    ("dve", "recip"): 204.0, ("dve", "reduce"): 296.0,
    ("act", "any"): 360.0,
    ("gps", "tt"): 440.0, ("gps", "ts"): 360.0,
}
XLAT = 100.0  # cross-engine semaphore latency


def classify(n):
    """Returns options = [(engine, cost), ...]. GPSIMD (Pool) supports only
    tensor_tensor and tensor_scalar; scalar_tensor_tensor is DVE-only."""
    if n.op == "sin" or n.op == "sqrt":
        return [("act", COST[("act", "any")])]
    if n.op == "recip":
        return [("dve", COST[("dve", "recip")])]
    if n.op == "square":
        return [("dve", COST[("dve", "tt")]), ("act", COST[("act", "any")]),
                ("gps", COST[("gps", "tt")])]
    if n.op in ("cadd", "cmul", "ts2"):
        # ts2 with non-(mult,add) pattern can't be an ACT Copy
        actok = True
        if n.op == "ts2" and (n.c[1], n.c[3]) != ("mult", "add"):
            actok = False
        opts = [("dve", COST[("dve", "ts")])]
        if actok:
            opts.append(("act", COST[("act", "any")]))
        opts.append(("gps", COST[("gps", "ts")]))
        return opts
    if n.op in ("add", "sub", "mul"):
        if isinstance(n.c, tuple) and n.c and n.c[0] == "stt_cmul":
            return [("dve", COST[("dve", "stt")])]
        return [("dve", COST[("dve", "tt")]), ("gps", COST[("gps", "tt")])]
    if n.op == "cmul_stt":  # cmul fused with mul/square arg
        return [("dve", COST[("dve", "stt")])]
    raise ValueError(n.op)


def plan(g, root):
    """STT fusion + ETF scheduling. Returns emit list ordered by virtual
    start time, with n.engine set."""
    # reachability + users
    reach = set()
    stack = [root]
    while stack:
        n = stack.pop()
        if n.id in reach:
            continue
        reach.add(n.id)
        stack.extend(n.args)
    for n in g.nodes:
        n.users = []
    order = [n for n in g.nodes if n.id in reach]
    for n in order:
        for a in n.args:
            a.users.append(n)

    # fusion: add/sub(x, cmul(c,y)) -> STT ; cmul(c, mul(x,y)/square(x)) -> STT
    for n in order:
        if n.op in ("add", "sub"):
            for k, a in enumerate(n.args):
                if a.op == "cmul" and len(a.users) == 1 and a.fused_into is None \
                        and a.args[0].fused_into is None \
                        and a.args[0].op != "const":
                    n.c = ("stt_cmul", k, a.c)
                    a.fused_into = n
                    break
        elif n.op == "cmul" and n.fused_into is None:
            a = n.args[0]
            if a.op in ("mul", "square") and len(a.users) == 1 \
                    and a.fused_into is None \
                    and all(aa.fused_into is None for aa in a.args):
                a.fused_into = n

    # effective deps of an emitted node (skipping fused producers)
    def deps(n):
        out = []
        for a in n.args:
            if a.fused_into is n:
                for aa in a.args:
                    if aa.op not in ("const", "in"):
                        out.append(aa)
            elif a.op not in ("const", "in"):
                out.append(a)
        return out

    emit_nodes = [n for n in order
                  if n.op not in ("const", "in") and n.fused_into is None]

    def opts_of(n):
        if n.op == "cmul" and n.args[0].fused_into is n:
            return classify(Expr("cmul_stt"))
        return classify(n)

    # critical-path priority (min cost per node)
    mincost = {n.id: min(c for _, c in opts_of(n)) for n in emit_nodes}
    prio = {}

    def get_prio(n):
        if n.id in prio:
            return prio[n.id]
        p = mincost[n.id] + max(
            (get_prio(u if u.fused_into is None else u.fused_into)
             for u in n.users if (u.fused_into is None or u.fused_into is not n)
             ), default=0.0)
        prio[n.id] = p
        return p

    import sys
    sys.setrecursionlimit(10000)
    for n in emit_nodes:
        n.prio = 0.0
    # compute prios iteratively in reverse topological order (nodes list is topo)
    for n in reversed(emit_nodes):
        best = 0.0
        for u in n.users:
            tgt = u.fused_into if u.fused_into is not None else u
            if tgt is n:
                continue
            if tgt.fused_into is None and tgt.op not in ("const", "in"):
                best = max(best, tgt.prio)
        n.prio = mincost[n.id] + best

    # ---- phase 1: static engine assignment (balance max load) ----
    # Critical-chain nodes keep their fastest engine; the rest greedily go to
    # the engine with the smallest resulting load.
    ndeps = {n.id: 0 for n in emit_nodes}
    dep_lists = {}
    for n in emit_nodes:
        dl = deps(n)
        dep_lists[n.id] = dl
        ndeps[n.id] = len(dl)
    users_emit = {n.id: [] for n in emit_nodes}
    for n in emit_nodes:
        for d in dep_lists[n.id]:
            users_emit[d.id].append(n)

    def run_etf(gamma, win, xlat):
        """ETF with load-penalty engine choice. Returns (makespan, sched:
        list of (n, engine, start, finish))."""
        nd = dict(ndeps)
        ready = [n for n in emit_nodes if nd[n.id] == 0]
        eng_free = {"dve": 0.0, "act": 0.0, "gps": 0.0}
        eload = {"dve": 0.0, "act": 0.0, "gps": 0.0}
        fin = {}
        eng_of = {}
        sched = []
        while ready:
            cands = []
            for n in ready:
                dr_cache = {}
                for e, c in opts_of(n):
                    dr = 0.0
                    for d in dep_lists[n.id]:
                        dr = max(dr, fin[d.id] +
                                 (xlat if eng_of[d.id] != e else 0.0))
                    st = max(eng_free[e], dr)
                    score = st + c + gamma * eload[e]
                    cands.append((score, n.prio, n, e, c, st))
            smin = min(c[0] for c in cands)
            _, _, n, e, c, st = max(
                (cd for cd in cands if cd[0] <= smin + win),
                key=lambda cd: (cd[1], -cd[0]))
            ready.remove(n)
            fin[n.id] = st + c
            eng_of[n.id] = e
            eng_free[e] = st + c
            eload[e] += c
            sched.append((n, e, st, st + c))
            for u in users_emit[n.id]:
                nd[u.id] -= 1
                if nd[u.id] == 0:
                    ready.append(u)
        return max(f for _, _, _, f in sched), sched

    best_ms, best_sched = None, None
    for gamma in (0.0, 0.02, 0.05, 0.1, 0.2, 0.4):
        for win in (0.0, 80.0, 150.0, 250.0):
            for xl in (100.0, 150.0):
                ms, sched = run_etf(gamma, win, xl)
                if best_ms is None or ms < best_ms:
                    best_ms, best_sched = ms, sched

    load = {"dve": 0.0, "act": 0.0, "gps": 0.0}
    for n, e, st, f in best_sched:
        n.engine = e
        n.start = st
        n.finish = f
        load[e] += f - st

    scheduled = [n for n, _, _, _ in best_sched]
    scheduled.sort(key=lambda n: (n.start, n.finish))
    for i2, n in enumerate(scheduled):
        n.order = i2
    makespan = best_ms
    return scheduled, load, makespan


# ----------------------------------------------------------------------------
# bass emission
# ----------------------------------------------------------------------------

NCORES = 8
B_FULL, H, CH = 2048, 64, 26
N_PER_CORE = B_FULL * H // NCORES          # 16384
P = 128
FD = N_PER_CORE // P                        # 128
NCH = 8


def _build_bass():
    import concourse.bass as bass
    from concourse.bacc import Bacc
    import concourse.mybir as mybir
    from concourse.tile import TileContext

    f32 = mybir.dt.float32
    alu = mybir.AluOpType
    AF = mybir.ActivationFunctionType

    g, root = build_graph()
    emit, load, makespan = plan(g, root)

    nc = Bacc()
    xs = nc.dram_tensor("xs", (P, NCH * FD), f32, kind="ExternalInput")
    out = nc.dram_tensor("out", (B_FULL // NCORES,), f32, kind="ExternalOutput")

    # liveness for slot allocation
    last_use = {}
    for n in emit:
        for a in n.args:
            if a.order is not None:
                last_use[a.id] = max(last_use.get(a.id, -1), n.order)
            if a.fused_into is n:
                for aa in a.args:
                    if aa.order is not None:
                        last_use[aa.id] = max(last_use.get(aa.id, -1), n.order)
    last_use[root.id] = len(emit) + 10

    with TileContext(nc) as tc:
        with tc.tile_pool(name="vals", bufs=1) as vp:
            # three staged input groups: [q2 q3 q4] [q1 q5] [hx hy hz],
            # issued at t=0 on three different HWDGE-capable engines so the
            # fixed DGE latencies overlap; transfers serialize on the DMA bus
            # in issue order (q2/q3/q4 first — head of the trig chain).
            stA = vp.tile([P, 3 * FD], f32, tag="stA", name="stA")
            stB = vp.tile([P, 2 * FD], f32, tag="stB", name="stB")
            stC = vp.tile([P, 3 * FD], f32, tag="stC", name="stC")
            nc.sync.dma_start(stA[:, :], xs[:, 0:3 * FD])
            nc.scalar.dma_start(stB[:, :], xs[:, 3 * FD:5 * FD])
            nc.gpsimd.dma_start(stC[:, :], xs[:, 5 * FD:8 * FD])
            # const APs for non-Copy activation biases (registered after the
            # DMAs so they don't delay them; barrier orders memset vs readers)
            for cv in (PI / 2,):
                t = nc.alloc_sbuf_tensor(f"constf32-{cv}", [128, 1], f32)
                nc.gpsimd.memset(t.ap(), cv)
                nc.const_aps.aps[(f32, float(cv))] = t.ap()
            nc.all_engine_barrier()

            def chan_ap(ch):
                if ch < 3:
                    return stA[:, ch * FD:(ch + 1) * FD]
                if ch < 5:
                    return stB[:, (ch - 3) * FD:(ch - 2) * FD]
                return stC[:, (ch - 5) * FD:(ch - 4) * FD]

            from collections import deque
            free_slots = deque()
            SLACK = 64
            n_slots = [0]
            node_tile = {}

            def ap_of(n):
                if n.op == "in":
                    return chan_ap(n.c)
                return node_tile[n.id][:, :]

            def alloc(n):
                if len(free_slots) > SLACK:
                    sl = free_slots.popleft()
                else:
                    sl = n_slots[0]
                    n_slots[0] += 1
                t = vp.tile([P, FD], f32, tag=f"s{sl}", name=f"v{n.id}", bufs=2)
                n.slot = sl
                node_tile[n.id] = t
                return t

            by_last = {}
            for nid, lu in last_use.items():
                by_last.setdefault(lu, []).append(nid)

            eng = {"dve": nc.vector, "act": nc.scalar, "gps": nc.gpsimd}
            ALU_OF = {"add": alu.add, "sub": alu.subtract, "mul": alu.mult}

            def emit_tt(e, engname, ot, a, b, op):
                e.tensor_tensor(ot, a, b, op)

            for n in emit:
                ot = alloc(n)[:, :]
                e = eng[n.engine]
                en = n.engine
                if n.op == "sin":
                    sc, b = n.c
                    nc.scalar.activation(ot, ap_of(n.args[0]), AF.Sin,
                                         bias=float(b), scale=float(sc))
                elif n.op == "sqrt":
                    nc.scalar.activation(ot, ap_of(n.args[0]), AF.Sqrt)
                elif n.op == "recip":
                    nc.vector.reciprocal_approx_fast(out=ot, in_=ap_of(n.args[0]))
                elif n.op == "square":
                    if en == "act":
                        nc.scalar.activation(ot, ap_of(n.args[0]), AF.Square)
                    else:
                        a = ap_of(n.args[0])
                        emit_tt(e, en, ot, a, a, alu.mult)
                elif n.op == "cadd":
                    if en == "act":
                        nc.scalar.activation(ot, ap_of(n.args[0]), AF.Copy,
                                             bias=float(n.c), scale=1.0)
                    else:
                        e.tensor_scalar_add(ot, ap_of(n.args[0]), float(n.c))
                elif n.op == "ts2":
                    s1, op0, s2, op1 = n.c
                    if en == "act":
                        nc.scalar.activation(ot, ap_of(n.args[0]), AF.Copy,
                                             bias=float(s2), scale=float(s1))
                    else:
                        e.tensor_scalar(ot, ap_of(n.args[0]), float(s1), float(s2),
                                        getattr(alu, op0), getattr(alu, op1))
                elif n.op == "cmul":
                    a = n.args[0]
                    if a.fused_into is n:
                        if a.op == "square":
                            x = yv = a.args[0]
                        else:
                            x, yv = a.args
                        e.scalar_tensor_tensor(ot, ap_of(x), float(n.c),
                                               ap_of(yv), alu.mult, alu.mult)
                    elif en == "act":
                        nc.scalar.activation(ot, ap_of(n.args[0]), AF.Copy,
                                             bias=0.0, scale=float(n.c))
                    else:
                        e.tensor_scalar_mul(ot, ap_of(n.args[0]), float(n.c))
                elif n.op in ("add", "sub"):
                    if isinstance(n.c, tuple) and n.c and n.c[0] == "stt_cmul":
                        _, k, cval = n.c
                        cm = n.args[k]
                        other = n.args[1 - k]
                        x = cm.args[0]
                        if n.op == "add":
                            e.scalar_tensor_tensor(ot, ap_of(x), float(cval),
                                                   ap_of(other), alu.mult, alu.add)
                        else:
                            if k == 1:
                                e.scalar_tensor_tensor(ot, ap_of(x), float(-cval),
                                                       ap_of(other), alu.mult,
                                                       alu.add)
                            else:
                                e.scalar_tensor_tensor(ot, ap_of(x), float(cval),
                                                       ap_of(other), alu.mult,
                                                       alu.subtract)
                    else:
                        emit_tt(e, en, ot, ap_of(n.args[0]), ap_of(n.args[1]),
                                ALU_OF[n.op])
                elif n.op == "mul":
                    emit_tt(e, en, ot, ap_of(n.args[0]), ap_of(n.args[1]),
                            alu.mult)
                else:
                    raise ValueError(n.op)

                for nid in by_last.get(n.order, []):
                    nd = g.nodes[nid]
                    if nd.slot is not None and nd.id != root.id:
                        free_slots.append(nd.slot)
                        nd.slot = None

            # epilogue: per-b sums (64-sample segments)
            osum = vp.tile([P, 2], f32, tag="osum", bufs=2)
            croot = node_tile[root.id]
            nc.vector.tensor_reduce(osum[:, 0:1], croot[:, 0:64],
                                    mybir.AxisListType.X, alu.add)
            nc.vector.tensor_reduce(osum[:, 1:2], croot[:, 64:128],
                                    mybir.AxisListType.X, alu.add)
            nc.gpsimd.dma_start(out.rearrange("(p j) -> p j", p=P), osum[:, :])

    nc.compile()
    return nc, len(emit), load, makespan


_CACHE = {}


def kernel(x, cond, time):
    from concourse.bass_utils import run_bass_kernel_spmd

    if "nc" not in _CACHE:
        nc, n_ops, load, makespan = _build_bass()
        _CACHE["nc"] = nc
        _CACHE["stats"] = (n_ops, load, makespan)
    nc = _CACHE["nc"]

    xf = np.ascontiguousarray(x, dtype=np.float32).reshape(B_FULL * H, CH)
    sel = xf[:, SRC_COLS]                                   # [131072, 8]
    in_maps = []
    for k in range(NCORES):
        shard = sel[k * N_PER_CORE:(k + 1) * N_PER_CORE]    # [16384, 8]
        # [128 p, 128 q, 8 c] -> [128, 8, 128] channel-major free layout
        arr = shard.reshape(P, FD, NCH).transpose(0, 2, 1)
        in_maps.append({"xs": np.ascontiguousarray(arr).reshape(P, NCH * FD)})
    res = run_bass_kernel_spmd(nc, in_maps, core_ids=list(range(NCORES)))
    _CACHE["exec_time_ns"] = res.exec_time_ns
    _CACHE["trace"] = res.instructions_and_trace
    outs = [res.results[k]["out"] for k in range(NCORES)]
    return np.concatenate(outs).astype(np.float32)


if __name__ == "__main__":
    g, root = build_graph()
    emit, load, makespan = plan(g, root)
    from collections import Counter
    print("emitted ops:", len(emit))
    print(Counter((n.engine, n.op) for n in emit))
    print("load est (us):", {k: v / 1000 for k, v in load.items()})
    print("virtual makespan (us):", makespan / 1000)
